# revision 3
# baseline (speedup 1.0000x reference)
"""Trainium2 Bass kernel for sparse CausalSelfAttention (8 full heads W=1024,
8 reduced-qk heads W=256), SPMD over 8 NeuronCores.

v3: phase-B/C interleaved per 512-T block, merged reduced-head q-blocks
(Q=512), trimmed per-key-tile query ranges with shared L/U strip masks,
packed reduced qk projection, bf16 datapath, engine-balanced copies.

Sharding: core c -> batch c//4, head-group g=c%4. Host sums the 4 c_proj
partials per batch element.
"""

import numpy as np

import concourse.bacc as bacc
import concourse.mybir as mybir
from concourse import bass_utils
from concourse.tile import TileContext

B, T, C = 2, 2048, 1024
HDIM = 64
RDIM = 32
WF, WR = 1024, 256
N_CORES = 8
NK = C // 128

F32 = mybir.dt.float32
BF16 = mybir.dt.bfloat16
DT = BF16
QF, QR = 512, 256


def _tiles(Q, W, i0):
    """Key-tiles for query block [i0, i0+Q): (kt, f_lo, f_hi, strips) with
    strips a list of ('L'|'U', col) in trimmed-range coords; a full-range
    tile is ordered first (clean one preferred)."""
    kt_lo = max(0, i0 - W + 1) // 128
    kt_hi = (i0 + Q - 1) // 128
    tiles = []
    for kt in range(kt_lo, kt_hi + 1):
        d = i0 - kt * 128
        f_lo = max(0, -d)
        f_hi = min(Q, W - d + 128)  # all d multiples of 128
        strips = []
        if d <= 0:
            strips.append(("L", 0))
        if d + f_hi - 1 >= W:
            strips.append(("U", f_hi - 128 - f_lo))
        tiles.append((kt, f_lo, f_hi, strips))
    full = [t for t in tiles if t[1] == 0 and t[2] == Q]
    head = next((t for t in full if not t[3]), full[0])
    return [head] + [t for t in tiles if t is not head]


def _emit_body(nc, pools, aps):
    wpool, qkpool, ppool, rpool, ps_ms, ps_y = pools
    xT, wq, wk, wqkr, wv, wproj, out = aps

    # ---- persistent tiles ----
    xall = qkpool.tile([128, NK, T], DT, tag="xall")
    wq_sb = wpool.tile([128, NK, 128], DT, tag="wq")
    wk_sb = wpool.tile([128, NK, 128], DT, tag="wk")
    wqkr_sb = wpool.tile([128, NK, 128], DT, tag="wqkr")
    wv_sb = wpool.tile([128, NK, 256], DT, tag="wv")
    wproj_sb = wpool.tile([128, 2, C], DT, tag="wproj")

    # triangular strip masks on gpsimd: L keeps u >= p, U keeps u < p
    mL = wpool.tile([128, 128], BF16, tag="mL")
    mU = wpool.tile([128, 128], BF16, tag="mU")
    nc.gpsimd.memset(mL[:], 1.0)
    nc.gpsimd.affine_select(out=mL, in_=mL, compare_op=mybir.AluOpType.is_ge,
                            fill=0.0, base=0, pattern=[[1, 128]],
                            channel_multiplier=-1)
    nc.gpsimd.memset(mU[:], 1.0)
    nc.gpsimd.affine_select(out=mU, in_=mU, compare_op=mybir.AluOpType.is_ge,
                            fill=0.0, base=-1, pattern=[[-1, 128]],
                            channel_multiplier=1)

    # transposed activations [dim-stack, T]
    qTf = qkpool.tile([128, T], DT, tag="qTf")  # rows: hA q (64) | hB q (64)
    kTf = qkpool.tile([128, T], DT, tag="kTf")
    qTr = qkpool.tile([128, T], DT, tag="qTr")  # rows: qrA|0|qrB|0
    kTr = qkpool.tile([128, T], DT, tag="kTr")  # rows: krA|0|krB|0
    for t_ in (qTr, kTr):  # zero pad rows: junk*0 could be NaN otherwise
        nc.gpsimd.memset(t_[32:64, :], 0.0)
        nc.gpsimd.memset(t_[96:128, :], 0.0)
    # v values + ones block: [128, T-tile, head, 128] (cols 64:128 = 1.0)
    v_sb = qkpool.tile([128, T // 128, 4, 128], BF16, tag="v")
    nc.gpsimd.memset(v_sb[:, :, :, 64:128], 1.0)
    yTf = qkpool.tile([128, T], DT, tag="yTf")
    yTr = qkpool.tile([128, T], DT, tag="yTr")

    def proj_block(tb):
        sl = slice(tb * 512, (tb + 1) * 512)
        if tb == 0:
            nc.sync.dma_start(wq_sb[:], wq.rearrange("p (k m) -> p k m", k=NK))
            for k in range(NK):
                nc.sync.dma_start(xall[:, k, sl], xT[:, k, sl])
            nc.sync.dma_start(wk_sb[:], wk.rearrange("p (k m) -> p k m", k=NK))
            nc.sync.dma_start(wqkr_sb[:],
                              wqkr.rearrange("p (k m) -> p k m", k=NK))
            nc.sync.dma_start(wv_sb[:], wv.rearrange("p (k m) -> p k m", k=NK))
            nc.sync.dma_start(xall[:, 0:4, 512:T], xT[:, 0:4, 512:T])
            nc.sync.dma_start(xall[:, 4:8, 512:T], xT[:, 4:8, 512:T])
            nc.sync.dma_start(wproj_sb[:],
                              wproj.rearrange("p (k m) -> p k m", k=2))
        for w_sb, dst in ((wq_sb, qTf), (wk_sb, kTf)):
            psum = ps_ms.tile([128, 512], F32, tag="m")
            for k in range(NK):
                nc.tensor.matmul(psum[:], w_sb[:, k, :], xall[:, k, sl],
                                 start=(k == 0), stop=(k == NK - 1))
            nc.vector.tensor_copy(dst[:, sl], psum[:])
        # packed reduced: psum rows [qrA|qrB|krA|krB] -> scatter copies
        psum = ps_ms.tile([128, 512], F32, tag="m")
        for k in range(NK):
            nc.tensor.matmul(psum[:], wqkr_sb[:, k, :], xall[:, k, sl],
                             start=(k == 0), stop=(k == NK - 1))
        nc.vector.tensor_copy(qTr[0:32, sl], psum[0:32, :])
        nc.vector.tensor_copy(qTr[64:96, sl], psum[32:64, :])
        nc.scalar.copy(kTr[0:32, sl], psum[64:96, :])
        nc.scalar.copy(kTr[64:96, sl], psum[96:128, :])
        for tt in range(4):
            gt = tb * 4 + tt
            psv = ps_ms.tile([128, 256], F32, tag="m")
            for k in range(NK):
                nc.tensor.matmul(psv[:], xall[:, k, gt * 128:(gt + 1) * 128],
                                 wv_sb[:, k, :],
                                 start=(k == 0), stop=(k == NK - 1))
            nc.scalar.copy(v_sb[:, gt, :, 0:64],
                           psv[:].rearrange("p (h d) -> p h d", h=4))

    def attn_block(qT, kT_, Q, W, heads, yT, qb):
        i0 = qb * Q
        tiles = _tiles(Q, W, i0)
        n = len(tiles)
        py_a = ps_y.tile([128, Q], F32, tag="yA")
        py_b = ps_y.tile([128, Q], F32, tag="yB")
        for idx, (kt, flo, fhi, strips) in enumerate(tiles):
            Qt = fhi - flo
            ksl = slice(kt * 128, (kt + 1) * 128)
            qsl = slice(i0 + flo, i0 + fhi)
            pss = ps_ms.tile([128, 2, 512], F32, tag="s")
            nc.tensor.matmul(pss[:, 0, 0:Qt], kT_[0:64, ksl], qT[0:64, qsl],
                             start=True, stop=True)
            nc.tensor.matmul(pss[:, 1, 0:Qt], kT_[64:128, ksl],
                             qT[64:128, qsl], start=True, stop=True)
            p_sb = ppool.tile([128, 2, 512], BF16, tag="p")
            nc.scalar.activation(p_sb[:, :, 0:Qt], pss[:, :, 0:Qt],
                                 mybir.ActivationFunctionType.Exp)
            for kind, so in strips:
                m = mL if kind == "L" else mU
                mm = m[:].rearrange("p (a u) -> p a u", a=1) \
                    .broadcast_to([128, 2, 128])
                nc.vector.tensor_mul(p_sb[:, :, so:so + 128],
                                     p_sb[:, :, so:so + 128], mm)
            nc.tensor.matmul(py_a[:, flo:fhi], v_sb[:, kt, heads[0], :],
                             p_sb[:, 0, 0:Qt],
                             start=(idx == 0), stop=(idx == n - 1))
            nc.tensor.matmul(py_b[:, flo:fhi], v_sb[:, kt, heads[1], :],
                             p_sb[:, 1, 0:Qt],
                             start=(idx == 0), stop=(idx == n - 1))
        for py, rows in ((py_a, slice(0, 64)), (py_b, slice(64, 128))):
            r_sb = rpool.tile([64, 512], F32, tag="r")
            nc.vector.reciprocal(r_sb[0:64, 0:Q], py[64:128, :])
            nc.vector.tensor_mul(yT[rows, i0:i0 + Q], py[0:64, :],
                                 r_sb[0:64, 0:Q])

    def c_proj(f):
        for tt in range(4 * f, 4 * f + 4):
            tsl = slice(tt * 128, (tt + 1) * 128)
            o_sb = ppool.tile([128, 1024], DT, tag="osb")
            for nb in range(2):
                nsl = slice(nb * 512, (nb + 1) * 512)
                pso = ps_ms.tile([128, 512], F32, tag="m")
                nc.tensor.matmul(pso[:], yTf[:, tsl], wproj_sb[:, 0, nsl],
                                 start=True, stop=False)
                nc.tensor.matmul(pso[:], yTr[:, tsl], wproj_sb[:, 1, nsl],
                                 start=False, stop=True)
                nc.vector.tensor_copy(o_sb[:, nsl], pso[:])
            nc.sync.dma_start(out[tsl, :], o_sb[:])

    # software pipeline: projections run one T-block ahead of attention,
    # c_proj one block behind, so attention stalls can be filled with
    # independent projection/c_proj work
    NF = T // QF
    proj_block(0)
    for f in range(NF):
        if f + 1 < NF:
            proj_block(f + 1)
        if f > 1:
            c_proj(f - 2)
        attn_block(qTf, kTf, QF, WF, (0, 1), yTf, f)
        for rqb in (2 * f, 2 * f + 1):
            attn_block(qTr, kTr, QR, WR, (2, 3), yTr, rqb)
    c_proj(NF - 2)
    c_proj(NF - 1)


def _build_nc(reps=1):
    nc = bacc.Bacc(trn_type="TRN2", target_bir_lowering=False, debug=False,
                   num_devices=1)

    xT = nc.dram_tensor("xT", [128, NK, T], DT, kind="ExternalInput").ap()
    wq = nc.dram_tensor("wq", [128, NK * 128], DT, kind="ExternalInput").ap()
    wk = nc.dram_tensor("wk", [128, NK * 128], DT, kind="ExternalInput").ap()
    wqkr = nc.dram_tensor("wqkr", [128, NK * 128], DT,
                          kind="ExternalInput").ap()
    wv = nc.dram_tensor("wv", [128, NK * 256], DT, kind="ExternalInput").ap()
    wproj = nc.dram_tensor("wproj", [128, 2 * C], DT,
                           kind="ExternalInput").ap()
    out = nc.dram_tensor("o", [T, C], DT, kind="ExternalOutput").ap()
    aps = (xT, wq, wk, wqkr, wv, wproj, out)

    with TileContext(nc) as tc:
        with (
            tc.tile_pool(name="wpool", bufs=1) as wpool,
            tc.tile_pool(name="qk", bufs=1) as qkpool,
            tc.tile_pool(name="ppool", bufs=8) as ppool,
            tc.tile_pool(name="rpool", bufs=4) as rpool,
            tc.tile_pool(name="ps_ms", bufs=2, space="PSUM") as ps_ms,
            tc.tile_pool(name="ps_y", bufs=1, space="PSUM") as ps_y,
        ):
            pools = (wpool, qkpool, ppool, rpool, ps_ms, ps_y)
            for _ in range(reps):
                _emit_body(nc, pools, aps)

    nc.compile()
    return nc


_NC_CACHE = {}


def _get_nc(reps=1):
    if reps not in _NC_CACHE:
        _NC_CACHE[reps] = _build_nc(reps)
    return _NC_CACHE[reps]


def _to_kmaj(w, cols):
    """[C, cols] -> [128, NK*cols] with k-tiles contiguous per partition."""
    return np.ascontiguousarray(
        w.reshape(NK, 128, cols).transpose(1, 0, 2).reshape(128, NK * cols))


def make_in_maps(x, w_qkv_full, w_qk_red, w_v_red, w_proj):
    import ml_dtypes
    bf = ml_dtypes.bfloat16
    x = np.asarray(x, np.float32)
    w_qkv_full = np.asarray(w_qkv_full, np.float32)
    w_qk_red = np.asarray(w_qk_red, np.float32)
    w_v_red = np.asarray(w_v_red, np.float32)
    w_proj = np.asarray(w_proj, np.float32)
    sf = np.float32(1.0 / np.sqrt(HDIM))
    sr = np.float32(1.0 / np.sqrt(RDIM))
    in_maps = []
    for c in range(N_CORES):
        b, g = divmod(c, 4)
        hA, hB = 2 * g, 2 * g + 1
        wqc = np.concatenate([w_qkv_full[:, 64 * hA:64 * hA + 64],
                              w_qkv_full[:, 64 * hB:64 * hB + 64]], 1) * sf
        wkc = np.concatenate([w_qkv_full[:, 512 + 64 * hA:512 + 64 * hA + 64],
                              w_qkv_full[:, 512 + 64 * hB:512 + 64 * hB + 64]],
                             1)
        wqkrc = np.concatenate(
            [w_qk_red[:, 32 * hA:32 * hA + 32] * sr,
             w_qk_red[:, 32 * hB:32 * hB + 32] * sr,
             w_qk_red[:, 256 + 32 * hA:256 + 32 * hA + 32],
             w_qk_red[:, 256 + 32 * hB:256 + 32 * hB + 32]], 1)
        wvc = np.concatenate([w_qkv_full[:, 1024 + 64 * hA:1024 + 64 * hA + 64],
                              w_qkv_full[:, 1024 + 64 * hB:1024 + 64 * hB + 64],
                              w_v_red[:, 64 * hA:64 * hA + 64],
                              w_v_red[:, 64 * hB:64 * hB + 64]], 1)
        wp = np.concatenate([w_proj[64 * hA:64 * hA + 64, :],
                             w_proj[64 * hB:64 * hB + 64, :],
                             w_proj[512 + 64 * hA:512 + 64 * hA + 64, :],
                             w_proj[512 + 64 * hB:512 + 64 * hB + 64, :]], 0)
        xk = np.ascontiguousarray(
            x[b].T.reshape(NK, 128, T).transpose(1, 0, 2))
        in_maps.append({
            "xT": xk.astype(bf),
            "wq": _to_kmaj(wqc, 128).astype(bf),
            "wk": _to_kmaj(wkc, 128).astype(bf),
            "wqkr": _to_kmaj(wqkrc, 128).astype(bf),
            "wv": _to_kmaj(wvc, 256).astype(bf),
            "wproj": np.ascontiguousarray(
                wp.reshape(2, 128, C).transpose(1, 0, 2)
                .reshape(128, 2 * C)).astype(bf),
        })
    return in_maps


def kernel(x, w_qkv_full, w_qk_red, w_v_red, w_proj):
    nc = _get_nc()
    in_maps = make_in_maps(x, w_qkv_full, w_qk_red, w_v_red, w_proj)
    r = bass_utils.run_bass_kernel_spmd(nc, in_maps,
                                        core_ids=list(range(N_CORES)),
                                        trace=False)
    outs = [np.asarray(r.results[c]["o"], np.float32)
            for c in range(N_CORES)]
    y = np.zeros((B, T, C), np.float32)
    for b in range(B):
        y[b] = outs[4 * b] + outs[4 * b + 1] + outs[4 * b + 2] + outs[4 * b + 3]
    return y
